# revision 31
# baseline (speedup 1.0000x reference)
"""Trainium2 Bass kernel for nn_AttentionModule (GNN message passing).

kernel(**inputs) takes the FULL unsharded inputs (as produced by
setup_inputs) and returns the FULL [B, 128] float32 output.

Strategy: data-parallel over graphs across 8 NeuronCores (batch is sorted, so
each core owns a contiguous range of graphs/nodes).  Per core, graphs are
packed into blocks of 32x128-node tiles with <= 18 graph slots; all segment
reductions are local matmuls against host-built slabs.

v2 math/layout notes (big tensors fp8/bf16, accumulations fp32):
  att = tanh(z), z = relu(x@fc1.T+b1)@fc2.T (+fc2_b==0)
  x2  = (1+att)*x = 2*sigma(2z)*x = 2*y2          (sigmoid trick)
  slab S holds 2.0 per member entry, so S-weighted sums of y2 give x2 sums:
    mean  = (y2n.T @ S) * (1/cnt)   tG = tanh(Wm.T @ mean)
    pd    = y2t.T @ tG (= dots/2);  c8 = S * sigma(2*pd);  out = y2n.T @ c8
  fc1: fp8 DoubleRow matmul for node-tiles 0-3 of each octet (PE rows 0:64)
       + two plain fp8 matmuls for tiles 4-7 (rows 64:128) -> 0.625 cyc/node
  fc2: two fp8 DoubleRow matmuls per octet with 64-deep contraction
       (rows 0:64 then 64:128) -> 0.5 cyc/node
  y2 transposes stage through dead PSUM regions (ph buf for even octets,
  pz cols 512:1024 for odd) so PSUM fits in exactly 8 banks.
"""

import sys
import numpy as np

sys.path.insert(0, "/opt/trn_rl_repo")

import ml_dtypes
from contextlib import ExitStack

import concourse.bass as bass
import concourse.bacc as bacc
import concourse.tile as tile
from concourse import mybir
from concourse.bass_utils import run_bass_kernel_spmd

BF = mybir.dt.bfloat16
F32 = mybir.dt.float32
FP8 = mybir.dt.float8e4
ALU = mybir.AluOpType
ACTF = mybir.ActivationFunctionType
DRM = mybir.MatmulPerfMode.DoubleRow
NPBF = ml_dtypes.bfloat16
NPF8 = ml_dtypes.float8_e4m3

NCORES = 8
D = 128
TBLK = 32          # 128-node tiles per block
GBLK = 18          # graph slots per block (data max is 17)

# engine-balance knobs
Y2M_DVE = (0, 2)   # octet % 4 in this set -> y2 multiply on DVE, else Pool
C8_POOL = True     # c8 mask-multiply on gpsimd
RELU_ACT = ()      # pair % 4 in this set -> relu on ACT, else DVE
DBG_PLAIN_FC1 = False
DBG_NO_PIPE = False
DBG_SEP_PH = False


class Cfg:
    def __init__(self, NB, TBLK=TBLK, GBLK=GBLK):
        self.NB = NB
        self.TBLK = TBLK
        self.GBLK = GBLK
        self.NTILES = NB * TBLK
        self.NNODES = self.NTILES * 128


# ---------------------------------------------------------------------------
# device program
# ---------------------------------------------------------------------------

def declare_io(nc, cfg):
    NB, GBLK = cfg.NB, cfg.GBLK
    d = {}
    d["xt"] = nc.dram_tensor("xt", [128, cfg.NTILES * 128], FP8, kind="ExternalInput").ap()
    d["xn"] = nc.dram_tensor("xn", [128, cfg.NTILES * 128], BF, kind="ExternalInput").ap()
    d["sl"] = nc.dram_tensor("sl", [128, cfg.NTILES * GBLK], FP8, kind="ExternalInput").ap()
    d["recip"] = nc.dram_tensor("recip", [128, NB * GBLK], F32, kind="ExternalInput").ap()
    d["fc1dr"] = nc.dram_tensor("fc1dr", [128, 128], FP8, kind="ExternalInput").ap()
    d["fc1drlo"] = nc.dram_tensor("fc1drlo", [128, 128], FP8, kind="ExternalInput").ap()
    d["fc1t"] = nc.dram_tensor("fc1t", [128, 32], BF, kind="ExternalInput").ap()
    d["fc2p"] = nc.dram_tensor("fc2p", [128, 512], BF, kind="ExternalInput").ap()
    d["wm"] = nc.dram_tensor("wm", [128, 128], F32, kind="ExternalInput").ap()
    d["b1"] = nc.dram_tensor("b1", [128, 1], F32, kind="ExternalInput").ap()
    d["ident"] = nc.dram_tensor("ident", [128, 128], BF, kind="ExternalInput").ap()
    d["outT"] = nc.dram_tensor("outT", [128, NB * GBLK], F32, kind="ExternalOutput").ap()
    return d


def build(tc, io, cfg):
    nc = tc.nc
    NB, GBLK = cfg.NB, cfg.GBLK
    G = GBLK

    with ExitStack() as ctx:
        ep = ctx.enter_context

        consts = ep(tc.tile_pool(name="consts", bufs=1))
        fc1dr = consts.tile([128, 128], FP8, tag="fc1dr")
        nc.sync.dma_start(fc1dr[:], io["fc1dr"])
        fc1drlo = consts.tile([128, 128], FP8, tag="fc1drlo")
        nc.sync.dma_start(fc1drlo[:], io["fc1drlo"])
        fc1t = consts.tile([128, 32], BF, tag="fc1t")
        nc.sync.dma_start(fc1t[:], io["fc1t"])
        fc2p = consts.tile([128, 512], BF, tag="fc2p")
        nc.sync.dma_start(fc2p[:], io["fc2p"])
        wm = consts.tile([128, 128], F32, tag="wm")
        nc.sync.dma_start(wm[:], io["wm"])
        b1c = consts.tile([128, 1], F32, tag="b1c")
        nc.sync.dma_start(b1c[:], io["b1"])
        ident = consts.tile([128, 128], BF, tag="ident")
        nc.sync.dma_start(ident[:], io["ident"])
        recip = consts.tile([128, NB * GBLK], F32, tag="recip")
        nc.sync.dma_start(recip[:], io["recip"])

        xtp = ep(tc.tile_pool(name="xt", bufs=3))
        xnp = ep(tc.tile_pool(name="xn", bufs=3))
        slp = ep(tc.tile_pool(name="sl", bufs=3))
        hp = ep(tc.tile_pool(name="h8", bufs=3))
        sigp = ep(tc.tile_pool(name="sig", bufs=3))
        y2np = ep(tc.tile_pool(name="y2n", bufs=3))
        y2tp = ep(tc.tile_pool(name="y2t", bufs=3))
        mtp = ep(tc.tile_pool(name="mt", bufs=2))
        tgp = ep(tc.tile_pool(name="tg", bufs=3))
        sdp = ep(tc.tile_pool(name="sd", bufs=2))
        cp = ep(tc.tile_pool(name="c8", bufs=2))
        outp = ep(tc.tile_pool(name="osb", bufs=2))

        # PSUM: pz 2x2 banks (fc1's ph lives in cols 0:256 of each pz buf),
        # pyt 2x1 banks, pmf 2x1 banks = 8 banks
        pzp = ep(tc.tile_pool(name="pz", bufs=2, space="PSUM"))
        pytp = ep(tc.tile_pool(name="pyt", bufs=1 if DBG_SEP_PH else 2, space="PSUM"))
        pmfp = ep(tc.tile_pool(name="pmf", bufs=2, space="PSUM"))
        php = ep(tc.tile_pool(name="phd", bufs=1, space="PSUM")) if DBG_SEP_PH else None

        # pmf layout per block: mean [0:G], tG [32:32+G],
        # fin0 [64:64+G], fin1 [96:96+G], pd [128:128+16*G]
        PD0 = 128

        def emit_tail_head(st):
            blk, pmf, y2t, y2n, ssb = st
            meant = mtp.tile([128, G], F32, tag="mt")
            nc.vector.tensor_tensor(
                meant[:], pmf[:, 0:G], recip[:, blk * G:(blk + 1) * G],
                op=ALU.mult)
            nc.tensor.matmul(pmf[:, 32:32 + G], wm[:], meant[:],
                             start=True, stop=True, skip_group_check=True)
            tgt = tgp.tile([128, G], BF, tag="tg")
            nc.scalar.activation(tgt[:], pmf[:, 32:32 + G], ACTF.Tanh)
            tgts[blk] = tgt

        def emit_tail_group(st, g):
            blk, pmf, y2t, y2n, ssb = st
            tgt = tgts[blk]
            for k in range(16):
                t = g * 16 + k
                nc.tensor.matmul(
                    pmf[:, PD0 + k * G:PD0 + (k + 1) * G],
                    y2t[:, t * 128:(t + 1) * 128], tgt[:],
                    start=True, stop=True, skip_group_check=True)
            sd = sdp.tile([128, 16 * G], BF, tag="sd")
            nc.scalar.activation(sd[:], pmf[:, PD0:PD0 + 16 * G],
                                 ACTF.Sigmoid, scale=2.0)
            c8 = cp.tile([128, 16 * G], BF, tag="c8")
            seng = nc.gpsimd if C8_POOL else nc.vector
            seng.tensor_tensor(
                c8[:], ssb[:, g * 16 * G:(g + 1) * 16 * G], sd[:], op=ALU.mult)
            fin0 = 64 if g == 0 else 96
            for k in range(16):
                t = g * 16 + k
                nc.tensor.matmul(
                    pmf[:, fin0:fin0 + G],
                    y2n[:, t * 128:(t + 1) * 128],
                    c8[:, k * G:(k + 1) * G],
                    start=(k == 0), stop=(k == 15), skip_group_check=True)

        def emit_tail_out(st):
            blk, pmf, y2t, y2n, ssb = st
            osb = outp.tile([128, G], F32, tag="osb")
            nc.vector.tensor_copy(osb[:], pmf[:, 64:64 + G])
            nc.vector.tensor_tensor(osb[:], osb[:], pmf[:, 96:96 + G], op=ALU.add)
            nc.scalar.dma_start(io["outT"][:, blk * G:(blk + 1) * G], osb[:])
            tgts.pop(blk)

        lwhi = fc1dr[:].rearrange("p (two m) -> p two m", two=2)
        lwlo = fc1drlo[:].rearrange("p (two m) -> p two m", two=2)

        def emit_pipe(po):
            """transposes + copy + mean matmuls for the pending octet."""
            o, y2n, y2t, ssb, pmf = po
            obase = (o % 4) * 1024
            pyt = pytp.tile([128, 1024], BF, tag="pyt")
            for t8 in range(8):
                t = (o % 4) * 8 + t8
                nc.tensor.transpose(
                    pyt[:, t8 * 128:(t8 + 1) * 128],
                    y2n[:, t * 128:(t + 1) * 128], ident[:])
            nc.vector.tensor_copy(y2t[:, obase:obase + 1024], pyt[:])
            for t8 in range(8):
                t = (o % 4) * 8 + t8
                nc.tensor.matmul(
                    pmf[:, 0:G],
                    y2n[:, t * 128:(t + 1) * 128],
                    ssb[:, t * G:(t + 1) * G],
                    start=(t == 0), stop=(t == TBLK - 1),
                    skip_group_check=True)

        tgts = {}
        prev = None          # previous block's tail state
        pend = None          # previous octet awaiting transposes
        for blk in range(NB):
            nbase = blk * TBLK * 128
            xt = xtp.tile([128, TBLK * 128], FP8, tag="xt")
            nc.sync.dma_start(xt[:], io["xt"][:, nbase:nbase + TBLK * 128])
            xn = xnp.tile([128, TBLK * 128], BF, tag="xn")
            nc.sync.dma_start(xn[:], io["xn"][:, nbase:nbase + TBLK * 128])
            ssb = slp.tile([128, TBLK * G], FP8, tag="sl")
            nc.sync.dma_start(ssb[:], io["sl"][:, blk * TBLK * G:(blk + 1) * TBLK * G])
            y2n = y2np.tile([128, TBLK * 128], BF, tag="y2n")
            y2t = y2tp.tile([128, TBLK * 128], BF, tag="y2t")
            pmf = pmfp.tile([128, 512], F32, tag="pmf")

            for o in range(4):
                obase = o * 1024
                pz = pzp.tile([128, 1024], F32, tag="pz")
                if DBG_SEP_PH:
                    phd = php.tile([128, 256], F32, tag="phd", name="phd")
                    phr = phd
                else:
                    phr = pz
                # fc1 into pz cols [0:256]: split DoubleRow (tiles 0-3) +
                # two plain bf16-weight matmuls (tiles 4-7)
                if DBG_PLAIN_FC1:
                    nc.tensor.matmul(phr[0:32, 0:256], fc1t[:],
                                     xt[:, obase:obase + 256],
                                     start=True, stop=True, tile_position=(0, 0),
                                     skip_group_check=True)
                    nc.tensor.matmul(phr[32:64, 0:256], fc1t[:],
                                     xt[:, obase + 256:obase + 512],
                                     start=True, stop=True, tile_position=(0, 32),
                                     skip_group_check=True)
                else:
                    xo = xt[:, obase:obase + 512].rearrange(
                        "p (t2 t k) -> p t2 t k", t2=2, t=2, k=128)
                    nc.tensor.matmul(phr[0:64, 0:256], lwhi, xo,
                                     start=True, stop=False, perf_mode=DRM,
                                     skip_group_check=True)
                    nc.tensor.matmul(phr[0:64, 0:256], lwlo, xo,
                                     start=False, stop=True, perf_mode=DRM,
                                     skip_group_check=True)
                nc.tensor.matmul(phr[64:96, 0:256], fc1t[:],
                                 xt[:, obase + 512:obase + 768],
                                 start=True, stop=True, tile_position=(0, 64),
                                 skip_group_check=True)
                nc.tensor.matmul(phr[96:128, 0:256], fc1t[:],
                                 xt[:, obase + 768:obase + 1024],
                                 start=True, stop=True, tile_position=(0, 96),
                                 skip_group_check=True)
                h8 = hp.tile([128, 256], BF, tag="h8")
                if (blk * 4 + o) % 4 in RELU_ACT:
                    nc.scalar.activation(h8[:], phr[:, 0:256], ACTF.Relu, bias=b1c[:])
                else:
                    nc.vector.tensor_scalar(h8[:], phr[:, 0:256], b1c[:], 0.0,
                                            op0=ALU.add, op1=ALU.max)
                # fc2: 2 plain 128-deep block-diagonal matmuls; xt's per-octet
                # tile permutation makes outputs land in natural node order:
                # t=0 -> nodes [0:512], t=1 -> [512:1024]
                for t in range(2):
                    nc.tensor.matmul(
                        pz[:, t * 512:(t + 1) * 512],
                        h8[:, t * 128:(t + 1) * 128], fc2p[:],
                        start=True, stop=True, skip_group_check=True)
                sig = sigp.tile([128, 1024], BF, tag="sig")
                nc.scalar.activation(sig[:], pz[:], ACTF.Sigmoid, scale=2.0)
                y2eng = nc.vector if o % 4 in Y2M_DVE else nc.gpsimd
                y2eng.tensor_tensor(
                    y2n[:, obase:obase + 1024],
                    xn[:, obase:obase + 1024], sig[:], op=ALU.mult)
                # pipelined transposes/copy/mean of the previous octet
                if DBG_NO_PIPE:
                    emit_pipe((blk * 4 + o, y2n, y2t, ssb, pmf))
                else:
                    if pend is not None:
                        emit_pipe(pend)
                    pend = (blk * 4 + o, y2n, y2t, ssb, pmf)
                # interleave previous block's tail
                if prev is not None:
                    if o == 1:
                        emit_tail_head(prev)
                    elif o == 2:
                        emit_tail_group(prev, 0)
                    elif o == 3:
                        emit_tail_group(prev, 1)
                        emit_tail_out(prev)
            prev = (blk, pmf, y2t, y2n, ssb)
        if not DBG_NO_PIPE:
            emit_pipe(pend)
        emit_tail_head(prev)
        emit_tail_group(prev, 0)
        emit_tail_group(prev, 1)
        emit_tail_out(prev)


# ---------------------------------------------------------------------------
# host-side prep / unshard
# ---------------------------------------------------------------------------

def plan_shards(batch_i32, B, ncores, tblk=TBLK, gblk=GBLK):
    cnt = np.bincount(batch_i32, minlength=B).astype(np.int64)
    starts = np.concatenate([[0], np.cumsum(cnt)])
    N = int(starts[-1])
    bounds = [0]
    for c in range(1, ncores):
        target = N * c // ncores
        g = int(np.searchsorted(starts, target))
        g = max(bounds[-1], min(g, B))
        bounds.append(g)
    bounds.append(B)
    cap = tblk * 128
    plans = []
    for c in range(ncores):
        glo, ghi = bounds[c], bounds[c + 1]
        blocks, cur, cur_nodes = [], [], 0
        for g in range(glo, ghi):
            n_g = int(cnt[g])
            assert n_g <= cap, f"graph {g} has {n_g} nodes > block capacity"
            if cur and (cur_nodes + n_g > cap or len(cur) >= gblk):
                blocks.append(cur)
                cur, cur_nodes = [], 0
            cur.append((g, int(starts[g]), n_g))
            cur_nodes += n_g
        if cur:
            blocks.append(cur)
        plans.append(blocks)
    NB = max(len(p) for p in plans)
    return plans, NB


XTPERM = [0, 4, 1, 5, 2, 6, 3, 7]   # xt tile order within each octet


def prep_core(x, plan, cfg):
    NB, TBLKc, GBLKc = cfg.NB, cfg.TBLK, cfg.GBLK
    xs = np.zeros((cfg.NNODES, D), np.float32)
    sl = np.zeros((cfg.NTILES * 128, GBLKc), NPF8)
    recip = np.zeros((NB, GBLKc), np.float32)
    meta = []
    for bi, blkg in enumerate(plan):
        pos = bi * TBLKc * 128
        for slot, (g, s, n_g) in enumerate(blkg):
            xs[pos:pos + n_g] = x[s:s + n_g]
            sl[pos:pos + n_g, slot] = NPF8(2.0)
            recip[bi, slot] = 1.0 / max(n_g, 1)
            meta.append((bi, slot, g))
            pos += n_g
    xperm = xs.reshape(cfg.NNODES // 1024, 8, 128, D)[:, XTPERM].reshape(
        cfg.NNODES, D)
    xt = np.ascontiguousarray(xperm.T.astype(NPF8))
    xn = np.ascontiguousarray(
        xs.astype(NPBF).reshape(cfg.NTILES, 128, D).transpose(1, 0, 2)
        .reshape(128, cfg.NTILES * D))
    sl_packed = np.ascontiguousarray(
        sl.reshape(cfg.NTILES, 128, GBLKc).transpose(1, 0, 2)
        .reshape(128, cfg.NTILES * GBLKc))
    recip_b = np.ascontiguousarray(
        np.broadcast_to(recip.reshape(1, NB * GBLKc), (128, NB * GBLKc)))
    return {"xt": xt, "xn": xn, "sl": sl_packed, "recip": recip_b}, meta


def prep_consts(Wm, fc1_w, fc1_b, fc2_w, fc2_b):
    assert np.allclose(np.asarray(fc2_b, np.float32), 0.0), \
        "nonzero fc2_b not supported by this kernel build"
    fc1 = np.asarray(fc1_w, np.float32)
    fc2 = np.asarray(fc2_w, np.float32)
    fc1dr = np.zeros((128, 2, 64), np.float32)
    for t2 in range(2):
        for a in range(32):
            fc1dr[:, t2, 32 * t2 + a] = fc1[a, :]
    fc1dr = fc1dr.reshape(128, 128)
    fc1dr_hi = fc1dr.astype(NPF8)
    fc1dr_lo = (fc1dr - fc1dr_hi.astype(np.float32)).astype(NPF8)
    fc1t = np.ascontiguousarray(fc1.T.astype(NPBF))
    fc2p = np.zeros((128, 512), np.float32)
    for j in range(4):
        for a in range(32):
            fc2p[32 * j + a, j * 128:(j + 1) * 128] = fc2[:, a]
    fc2p = np.ascontiguousarray(fc2p.astype(NPBF))
    b1 = np.tile(np.asarray(fc1_b, np.float32), 4).reshape(128, 1)
    wm = np.ascontiguousarray(np.asarray(Wm, np.float32))
    ident = np.eye(128, dtype=NPBF)
    return {"fc1dr": np.ascontiguousarray(fc1dr_hi),
            "fc1drlo": np.ascontiguousarray(fc1dr_lo),
            "fc1t": fc1t, "fc2p": fc2p, "wm": wm,
            "b1": np.ascontiguousarray(b1), "ident": ident}


def unshard(outTs, metas, B, cfg):
    out = np.zeros((B, D), np.float32)
    for outT, meta in zip(outTs, metas):
        cols = [bi * cfg.GBLK + slot for (bi, slot, g) in meta]
        gs = [g for (bi, slot, g) in meta]
        out[gs] = outT[:, cols].T
    return out


# ---------------------------------------------------------------------------
# top-level entry
# ---------------------------------------------------------------------------

_CACHE = {}


def _get_program(NB):
    key = (NB, TBLK, GBLK)
    if key not in _CACHE:
        nc = bacc.Bacc("TRN2", target_bir_lowering=False, debug=False,
                       num_devices=NCORES)
        cfg = Cfg(NB)
        io = declare_io(nc, cfg)
        with tile.TileContext(nc) as tc:
            build(tc, io, cfg)
        nc.compile()
        _CACHE[key] = (nc, cfg)
    return _CACHE[key]


def _run(inputs, trace=False):
    x = np.asarray(inputs["x"], np.float32)
    batch = np.asarray(inputs["batch"]).astype(np.int32)
    B = int(np.asarray(inputs["size"]))
    plans, NB = plan_shards(batch, B, NCORES)
    nc, cfg = _get_program(NB)
    consts = prep_consts(inputs["Wm"], inputs["fc1_w"], inputs["fc1_b"],
                         inputs["fc2_w"], inputs["fc2_b"])
    in_maps, metas = [], []
    for c in range(NCORES):
        core_in, meta = prep_core(x, plans[c], cfg)
        core_in.update(consts)
        in_maps.append(core_in)
        metas.append(meta)
    res = run_bass_kernel_spmd(nc, in_maps, core_ids=list(range(NCORES)),
                               trace=trace)
    outTs = [res.results[c]["outT"] for c in range(NCORES)]
    out = unshard(outTs, metas, B, cfg)
    return out, res


def kernel(**inputs):
    out, _ = _run(inputs, trace=False)
    return out


# revision 32
# speedup vs baseline: 1.2090x; 1.2090x over previous
"""Trainium2 Bass kernel for nn_AttentionModule (GNN message passing).

kernel(**inputs) takes the FULL unsharded inputs (as produced by
setup_inputs) and returns the FULL [B, 128] float32 output.

Strategy: data-parallel over graphs across 8 NeuronCores (batch is sorted, so
each core owns a contiguous range of graphs/nodes).  Per core, graphs are
packed into blocks of 32x128-node tiles with <= 18 graph slots; all segment
reductions are local matmuls against host-built slabs.

v2 math/layout notes (big tensors fp8/bf16, accumulations fp32):
  att = tanh(z), z = relu(x@fc1.T+b1)@fc2.T (+fc2_b==0)
  x2  = (1+att)*x = 2*sigma(2z)*x = 2*y2          (sigmoid trick)
  slab S holds 2.0 per member entry, so S-weighted sums of y2 give x2 sums:
    mean  = (y2n.T @ S) * (1/cnt)   tG = tanh(Wm.T @ mean)
    pd    = y2t.T @ tG (= dots/2);  c8 = S * sigma(2*pd);  out = y2n.T @ c8
  fc1: fp8 DoubleRow matmul for node-tiles 0-3 of each octet (PE rows 0:64)
       + two plain fp8 matmuls for tiles 4-7 (rows 64:128) -> 0.625 cyc/node
  fc2: two fp8 DoubleRow matmuls per octet with 64-deep contraction
       (rows 0:64 then 64:128) -> 0.5 cyc/node
  y2 transposes stage through dead PSUM regions (ph buf for even octets,
  pz cols 512:1024 for odd) so PSUM fits in exactly 8 banks.
"""

import sys
import numpy as np

sys.path.insert(0, "/opt/trn_rl_repo")

import ml_dtypes
from contextlib import ExitStack

import concourse.bass as bass
import concourse.bacc as bacc
import concourse.tile as tile
from concourse import mybir
from concourse.bass_utils import run_bass_kernel_spmd

BF = mybir.dt.bfloat16
F32 = mybir.dt.float32
FP8 = mybir.dt.float8e4
ALU = mybir.AluOpType
ACTF = mybir.ActivationFunctionType
DRM = mybir.MatmulPerfMode.DoubleRow
NPBF = ml_dtypes.bfloat16
NPF8 = ml_dtypes.float8_e4m3

NCORES = 8
D = 128
TBLK = 32          # 128-node tiles per block
GBLK = 18          # graph slots per block (data max is 17)

# engine-balance knobs
Y2M_DVE = (0, 1, 2, 3)   # octet % 4 in this set -> y2 multiply on DVE, else Pool
C8_POOL = True     # c8 mask-multiply on gpsimd
RELU_ACT = ()      # pair % 4 in this set -> relu on ACT, else DVE
DBG_PLAIN_FC1 = False
DBG_NO_PIPE = False
DBG_SEP_PH = False


class Cfg:
    def __init__(self, NB, TBLK=TBLK, GBLK=GBLK):
        self.NB = NB
        self.TBLK = TBLK
        self.GBLK = GBLK
        self.NTILES = NB * TBLK
        self.NNODES = self.NTILES * 128


# ---------------------------------------------------------------------------
# device program
# ---------------------------------------------------------------------------

def declare_io(nc, cfg):
    NB, GBLK = cfg.NB, cfg.GBLK
    d = {}
    d["xt"] = nc.dram_tensor("xt", [128, cfg.NTILES * 128], FP8, kind="ExternalInput").ap()
    d["xn"] = nc.dram_tensor("xn", [128, cfg.NTILES * 128], BF, kind="ExternalInput").ap()
    d["sl"] = nc.dram_tensor("sl", [128, cfg.NTILES * GBLK], FP8, kind="ExternalInput").ap()
    d["recip"] = nc.dram_tensor("recip", [128, NB * GBLK], F32, kind="ExternalInput").ap()
    d["fc1dr"] = nc.dram_tensor("fc1dr", [128, 128], FP8, kind="ExternalInput").ap()
    d["fc1drlo"] = nc.dram_tensor("fc1drlo", [128, 128], FP8, kind="ExternalInput").ap()
    d["fc1t"] = nc.dram_tensor("fc1t", [128, 32], BF, kind="ExternalInput").ap()
    d["fc2p"] = nc.dram_tensor("fc2p", [128, 512], BF, kind="ExternalInput").ap()
    d["wm"] = nc.dram_tensor("wm", [128, 128], F32, kind="ExternalInput").ap()
    d["b1"] = nc.dram_tensor("b1", [128, 1], F32, kind="ExternalInput").ap()
    d["ident"] = nc.dram_tensor("ident", [128, 128], BF, kind="ExternalInput").ap()
    d["outT"] = nc.dram_tensor("outT", [128, NB * GBLK], F32, kind="ExternalOutput").ap()
    return d


def build(tc, io, cfg):
    nc = tc.nc
    NB, GBLK = cfg.NB, cfg.GBLK
    G = GBLK

    with ExitStack() as ctx:
        ep = ctx.enter_context

        consts = ep(tc.tile_pool(name="consts", bufs=1))
        fc1dr = consts.tile([128, 128], FP8, tag="fc1dr")
        nc.sync.dma_start(fc1dr[:], io["fc1dr"])
        fc1drlo = consts.tile([128, 128], FP8, tag="fc1drlo")
        nc.sync.dma_start(fc1drlo[:], io["fc1drlo"])
        fc1t = consts.tile([128, 32], BF, tag="fc1t")
        nc.sync.dma_start(fc1t[:], io["fc1t"])
        fc2p = consts.tile([128, 512], BF, tag="fc2p")
        nc.sync.dma_start(fc2p[:], io["fc2p"])
        wm = consts.tile([128, 128], F32, tag="wm")
        nc.sync.dma_start(wm[:], io["wm"])
        b1c = consts.tile([128, 1], F32, tag="b1c")
        nc.sync.dma_start(b1c[:], io["b1"])
        ident = consts.tile([128, 128], BF, tag="ident")
        nc.sync.dma_start(ident[:], io["ident"])
        recip = consts.tile([128, NB * GBLK], F32, tag="recip")
        nc.sync.dma_start(recip[:], io["recip"])

        xtp = ep(tc.tile_pool(name="xt", bufs=3))
        xnp = ep(tc.tile_pool(name="xn", bufs=3))
        slp = ep(tc.tile_pool(name="sl", bufs=3))
        hp = ep(tc.tile_pool(name="h8", bufs=3))
        sigp = ep(tc.tile_pool(name="sig", bufs=3))
        y2np = ep(tc.tile_pool(name="y2n", bufs=3))
        y2tp = ep(tc.tile_pool(name="y2t", bufs=3))
        mtp = ep(tc.tile_pool(name="mt", bufs=2))
        tgp = ep(tc.tile_pool(name="tg", bufs=3))
        sdp = ep(tc.tile_pool(name="sd", bufs=2))
        cp = ep(tc.tile_pool(name="c8", bufs=2))
        outp = ep(tc.tile_pool(name="osb", bufs=2))

        # PSUM: pz 2x2 banks (fc1's ph lives in cols 0:256 of each pz buf),
        # pyt 2x1 banks, pmf 2x1 banks = 8 banks
        pzp = ep(tc.tile_pool(name="pz", bufs=2, space="PSUM"))
        pytp = ep(tc.tile_pool(name="pyt", bufs=1 if DBG_SEP_PH else 2, space="PSUM"))
        pmfp = ep(tc.tile_pool(name="pmf", bufs=2, space="PSUM"))
        php = ep(tc.tile_pool(name="phd", bufs=1, space="PSUM")) if DBG_SEP_PH else None

        # pmf layout per block: mean [0:G], tG [32:32+G],
        # fin0 [64:64+G], fin1 [96:96+G], pd [128:128+16*G]
        PD0 = 128

        def emit_tail_head(st):
            blk, pmf, y2t, y2n, ssb = st
            meant = mtp.tile([128, G], F32, tag="mt")
            nc.vector.tensor_tensor(
                meant[:], pmf[:, 0:G], recip[:, blk * G:(blk + 1) * G],
                op=ALU.mult)
            nc.tensor.matmul(pmf[:, 32:32 + G], wm[:], meant[:],
                             start=True, stop=True, skip_group_check=True)
            tgt = tgp.tile([128, G], BF, tag="tg")
            nc.scalar.activation(tgt[:], pmf[:, 32:32 + G], ACTF.Tanh)
            tgts[blk] = tgt

        def emit_tail_group(st, g):
            blk, pmf, y2t, y2n, ssb = st
            tgt = tgts[blk]
            for k in range(16):
                t = g * 16 + k
                nc.tensor.matmul(
                    pmf[:, PD0 + k * G:PD0 + (k + 1) * G],
                    y2t[:, t * 128:(t + 1) * 128], tgt[:],
                    start=True, stop=True, skip_group_check=True)
            sd = sdp.tile([128, 16 * G], BF, tag="sd")
            nc.scalar.activation(sd[:], pmf[:, PD0:PD0 + 16 * G],
                                 ACTF.Sigmoid, scale=2.0)
            c8 = cp.tile([128, 16 * G], BF, tag="c8")
            seng = nc.gpsimd if C8_POOL else nc.vector
            seng.tensor_tensor(
                c8[:], ssb[:, g * 16 * G:(g + 1) * 16 * G], sd[:], op=ALU.mult)
            fin0 = 64 if g == 0 else 96
            for k in range(16):
                t = g * 16 + k
                nc.tensor.matmul(
                    pmf[:, fin0:fin0 + G],
                    y2n[:, t * 128:(t + 1) * 128],
                    c8[:, k * G:(k + 1) * G],
                    start=(k == 0), stop=(k == 15), skip_group_check=True)

        def emit_tail_out(st):
            blk, pmf, y2t, y2n, ssb = st
            osb = outp.tile([128, G], F32, tag="osb")
            nc.vector.tensor_copy(osb[:], pmf[:, 64:64 + G])
            nc.vector.tensor_tensor(osb[:], osb[:], pmf[:, 96:96 + G], op=ALU.add)
            nc.scalar.dma_start(io["outT"][:, blk * G:(blk + 1) * G], osb[:])
            tgts.pop(blk)

        lwhi = fc1dr[:].rearrange("p (two m) -> p two m", two=2)
        lwlo = fc1drlo[:].rearrange("p (two m) -> p two m", two=2)

        def emit_pipe(po):
            """transposes + copy + mean matmuls for the pending octet."""
            o, y2n, y2t, ssb, pmf = po
            obase = (o % 4) * 1024
            pyt = pytp.tile([128, 1024], BF, tag="pyt")
            for t8 in range(8):
                t = (o % 4) * 8 + t8
                nc.tensor.transpose(
                    pyt[:, t8 * 128:(t8 + 1) * 128],
                    y2n[:, t * 128:(t + 1) * 128], ident[:])
            nc.vector.tensor_copy(y2t[:, obase:obase + 1024], pyt[:])
            for t8 in range(8):
                t = (o % 4) * 8 + t8
                nc.tensor.matmul(
                    pmf[:, 0:G],
                    y2n[:, t * 128:(t + 1) * 128],
                    ssb[:, t * G:(t + 1) * G],
                    start=(t == 0), stop=(t == TBLK - 1),
                    skip_group_check=True)

        tgts = {}
        prev = None          # previous block's tail state
        pend = None          # previous octet awaiting transposes
        for blk in range(NB):
            nbase = blk * TBLK * 128
            xt = xtp.tile([128, TBLK * 128], FP8, tag="xt")
            nc.sync.dma_start(xt[:], io["xt"][:, nbase:nbase + TBLK * 128])
            xn = xnp.tile([128, TBLK * 128], BF, tag="xn")
            nc.sync.dma_start(xn[:], io["xn"][:, nbase:nbase + TBLK * 128])
            ssb = slp.tile([128, TBLK * G], FP8, tag="sl")
            nc.sync.dma_start(ssb[:], io["sl"][:, blk * TBLK * G:(blk + 1) * TBLK * G])
            y2n = y2np.tile([128, TBLK * 128], BF, tag="y2n")
            y2t = y2tp.tile([128, TBLK * 128], BF, tag="y2t")
            pmf = pmfp.tile([128, 512], F32, tag="pmf")

            for o in range(4):
                obase = o * 1024
                pz = pzp.tile([128, 1024], F32, tag="pz")
                if DBG_SEP_PH:
                    phd = php.tile([128, 256], F32, tag="phd", name="phd")
                    phr = phd
                else:
                    phr = pz
                # fc1 into pz cols [0:256]: split DoubleRow (tiles 0-3) +
                # two plain bf16-weight matmuls (tiles 4-7)
                if DBG_PLAIN_FC1:
                    nc.tensor.matmul(phr[0:32, 0:256], fc1t[:],
                                     xt[:, obase:obase + 256],
                                     start=True, stop=True, tile_position=(0, 0),
                                     skip_group_check=True)
                    nc.tensor.matmul(phr[32:64, 0:256], fc1t[:],
                                     xt[:, obase + 256:obase + 512],
                                     start=True, stop=True, tile_position=(0, 32),
                                     skip_group_check=True)
                else:
                    xo = xt[:, obase:obase + 512].rearrange(
                        "p (t2 t k) -> p t2 t k", t2=2, t=2, k=128)
                    nc.tensor.matmul(phr[0:64, 0:256], lwhi, xo,
                                     start=True, stop=False, perf_mode=DRM,
                                     skip_group_check=True)
                    nc.tensor.matmul(phr[0:64, 0:256], lwlo, xo,
                                     start=False, stop=True, perf_mode=DRM,
                                     skip_group_check=True)
                nc.tensor.matmul(phr[64:96, 0:256], fc1t[:],
                                 xt[:, obase + 512:obase + 768],
                                 start=True, stop=True, tile_position=(0, 64),
                                 skip_group_check=True)
                nc.tensor.matmul(phr[96:128, 0:256], fc1t[:],
                                 xt[:, obase + 768:obase + 1024],
                                 start=True, stop=True, tile_position=(0, 96),
                                 skip_group_check=True)
                h8 = hp.tile([128, 256], BF, tag="h8")
                if (blk * 4 + o) % 4 in RELU_ACT:
                    nc.scalar.activation(h8[:], phr[:, 0:256], ACTF.Relu, bias=b1c[:])
                else:
                    nc.vector.tensor_scalar(h8[:], phr[:, 0:256], b1c[:], 0.0,
                                            op0=ALU.add, op1=ALU.max)
                # fc2: 2 plain 128-deep block-diagonal matmuls; xt's per-octet
                # tile permutation makes outputs land in natural node order:
                # t=0 -> nodes [0:512], t=1 -> [512:1024]
                for t in range(2):
                    nc.tensor.matmul(
                        pz[:, t * 512:(t + 1) * 512],
                        h8[:, t * 128:(t + 1) * 128], fc2p[:],
                        start=True, stop=True, skip_group_check=True)
                sig = sigp.tile([128, 1024], BF, tag="sig")
                nc.scalar.activation(sig[:], pz[:], ACTF.Sigmoid, scale=2.0)
                y2eng = nc.vector if o % 4 in Y2M_DVE else nc.gpsimd
                y2eng.tensor_tensor(
                    y2n[:, obase:obase + 1024],
                    xn[:, obase:obase + 1024], sig[:], op=ALU.mult)
                # pipelined transposes/copy/mean of the previous octet
                if DBG_NO_PIPE:
                    emit_pipe((blk * 4 + o, y2n, y2t, ssb, pmf))
                else:
                    if pend is not None:
                        emit_pipe(pend)
                    pend = (blk * 4 + o, y2n, y2t, ssb, pmf)
                # interleave previous block's tail
                if prev is not None:
                    if o == 1:
                        emit_tail_head(prev)
                    elif o == 2:
                        emit_tail_group(prev, 0)
                    elif o == 3:
                        emit_tail_group(prev, 1)
                        emit_tail_out(prev)
            prev = (blk, pmf, y2t, y2n, ssb)
        if not DBG_NO_PIPE:
            emit_pipe(pend)
        emit_tail_head(prev)
        emit_tail_group(prev, 0)
        emit_tail_group(prev, 1)
        emit_tail_out(prev)


# ---------------------------------------------------------------------------
# host-side prep / unshard
# ---------------------------------------------------------------------------

def plan_shards(batch_i32, B, ncores, tblk=TBLK, gblk=GBLK):
    cnt = np.bincount(batch_i32, minlength=B).astype(np.int64)
    starts = np.concatenate([[0], np.cumsum(cnt)])
    N = int(starts[-1])
    bounds = [0]
    for c in range(1, ncores):
        target = N * c // ncores
        g = int(np.searchsorted(starts, target))
        g = max(bounds[-1], min(g, B))
        bounds.append(g)
    bounds.append(B)
    cap = tblk * 128
    plans = []
    for c in range(ncores):
        glo, ghi = bounds[c], bounds[c + 1]
        blocks, cur, cur_nodes = [], [], 0
        for g in range(glo, ghi):
            n_g = int(cnt[g])
            assert n_g <= cap, f"graph {g} has {n_g} nodes > block capacity"
            if cur and (cur_nodes + n_g > cap or len(cur) >= gblk):
                blocks.append(cur)
                cur, cur_nodes = [], 0
            cur.append((g, int(starts[g]), n_g))
            cur_nodes += n_g
        if cur:
            blocks.append(cur)
        plans.append(blocks)
    NB = max(len(p) for p in plans)
    return plans, NB


XTPERM = [0, 4, 1, 5, 2, 6, 3, 7]   # xt tile order within each octet


def prep_core(x, plan, cfg):
    NB, TBLKc, GBLKc = cfg.NB, cfg.TBLK, cfg.GBLK
    xs = np.zeros((cfg.NNODES, D), np.float32)
    sl = np.zeros((cfg.NTILES * 128, GBLKc), NPF8)
    recip = np.zeros((NB, GBLKc), np.float32)
    meta = []
    for bi, blkg in enumerate(plan):
        pos = bi * TBLKc * 128
        for slot, (g, s, n_g) in enumerate(blkg):
            xs[pos:pos + n_g] = x[s:s + n_g]
            sl[pos:pos + n_g, slot] = NPF8(2.0)
            recip[bi, slot] = 1.0 / max(n_g, 1)
            meta.append((bi, slot, g))
            pos += n_g
    xperm = xs.reshape(cfg.NNODES // 1024, 8, 128, D)[:, XTPERM].reshape(
        cfg.NNODES, D)
    xt = np.ascontiguousarray(xperm.T.astype(NPF8))
    xn = np.ascontiguousarray(
        xs.astype(NPBF).reshape(cfg.NTILES, 128, D).transpose(1, 0, 2)
        .reshape(128, cfg.NTILES * D))
    sl_packed = np.ascontiguousarray(
        sl.reshape(cfg.NTILES, 128, GBLKc).transpose(1, 0, 2)
        .reshape(128, cfg.NTILES * GBLKc))
    recip_b = np.ascontiguousarray(
        np.broadcast_to(recip.reshape(1, NB * GBLKc), (128, NB * GBLKc)))
    return {"xt": xt, "xn": xn, "sl": sl_packed, "recip": recip_b}, meta


def prep_consts(Wm, fc1_w, fc1_b, fc2_w, fc2_b):
    assert np.allclose(np.asarray(fc2_b, np.float32), 0.0), \
        "nonzero fc2_b not supported by this kernel build"
    fc1 = np.asarray(fc1_w, np.float32)
    fc2 = np.asarray(fc2_w, np.float32)
    fc1dr = np.zeros((128, 2, 64), np.float32)
    for t2 in range(2):
        for a in range(32):
            fc1dr[:, t2, 32 * t2 + a] = fc1[a, :]
    fc1dr = fc1dr.reshape(128, 128)
    fc1dr_hi = fc1dr.astype(NPF8)
    fc1dr_lo = (fc1dr - fc1dr_hi.astype(np.float32)).astype(NPF8)
    fc1t = np.ascontiguousarray(fc1.T.astype(NPBF))
    fc2p = np.zeros((128, 512), np.float32)
    for j in range(4):
        for a in range(32):
            fc2p[32 * j + a, j * 128:(j + 1) * 128] = fc2[:, a]
    fc2p = np.ascontiguousarray(fc2p.astype(NPBF))
    b1 = np.tile(np.asarray(fc1_b, np.float32), 4).reshape(128, 1)
    wm = np.ascontiguousarray(np.asarray(Wm, np.float32))
    ident = np.eye(128, dtype=NPBF)
    return {"fc1dr": np.ascontiguousarray(fc1dr_hi),
            "fc1drlo": np.ascontiguousarray(fc1dr_lo),
            "fc1t": fc1t, "fc2p": fc2p, "wm": wm,
            "b1": np.ascontiguousarray(b1), "ident": ident}


def unshard(outTs, metas, B, cfg):
    out = np.zeros((B, D), np.float32)
    for outT, meta in zip(outTs, metas):
        cols = [bi * cfg.GBLK + slot for (bi, slot, g) in meta]
        gs = [g for (bi, slot, g) in meta]
        out[gs] = outT[:, cols].T
    return out


# ---------------------------------------------------------------------------
# top-level entry
# ---------------------------------------------------------------------------

_CACHE = {}


def _get_program(NB):
    key = (NB, TBLK, GBLK)
    if key not in _CACHE:
        nc = bacc.Bacc("TRN2", target_bir_lowering=False, debug=False,
                       num_devices=NCORES)
        cfg = Cfg(NB)
        io = declare_io(nc, cfg)
        with tile.TileContext(nc) as tc:
            build(tc, io, cfg)
        nc.compile()
        _CACHE[key] = (nc, cfg)
    return _CACHE[key]


def _run(inputs, trace=False):
    x = np.asarray(inputs["x"], np.float32)
    batch = np.asarray(inputs["batch"]).astype(np.int32)
    B = int(np.asarray(inputs["size"]))
    plans, NB = plan_shards(batch, B, NCORES)
    nc, cfg = _get_program(NB)
    consts = prep_consts(inputs["Wm"], inputs["fc1_w"], inputs["fc1_b"],
                         inputs["fc2_w"], inputs["fc2_b"])
    in_maps, metas = [], []
    for c in range(NCORES):
        core_in, meta = prep_core(x, plans[c], cfg)
        core_in.update(consts)
        in_maps.append(core_in)
        metas.append(meta)
    res = run_bass_kernel_spmd(nc, in_maps, core_ids=list(range(NCORES)),
                               trace=trace)
    outTs = [res.results[c]["outT"] for c in range(NCORES)]
    out = unshard(outTs, metas, B, cfg)
    return out, res


def kernel(**inputs):
    out, _ = _run(inputs, trace=False)
    return out
